# revision 4
# baseline (speedup 1.0000x reference)
"""Cepstrum -> minimum-phase impulse response on 8 Trainium2 NeuronCores.

Math: the reference recurrence  n*h_n = sum_k (k c_k) h_{n-k}, h_0 = exp(c_0)
is exactly the power-series exponential h = exp(C(z)) mod z^512 for the
degree-255 polynomial C. We evaluate it spectrally with a length-512 DFT:

    h = IDFT_512( exp( DFT_512(c) ) )

Aliasing folds h_{n+512} into h_n; since order-2 products of C reach only
degree 510, the alias is order-3 (~1e-4 abs, validated on host in f64).

Device pipeline per 512-row supertile:
  - fwd:  R/I[bins, rows] via ONE fp8 DoubleRow matmul each per 128-bin
          chunk (K=256 contraction in a single pass, operands host-quantized)
  - ACT:  A = exp(R), Sn = sin(I), Cs = cos(I) on [128,1024] tiles
          (single table set: exp_and_friends, Sin lowered to Sin2pi)
  - DVE:  HRe = A*Cs, HIm = A*Sn (fp16)
  - the per-row DC term h0n = exp(sum c)/512 rides a free matmul slot: the
    Nyquist-bin sin row is identically zero, so si[255,:] := 1 and
    HIm[Nyquist,:] := h0n (small per-supertile DMA overwrite)
  - inv:  h[rows, n] += HRe/HIm-stationary fp16 matmuls over ci/si
  - out:  plain PSUM -> fp16 copy (split scalar/vector engines), DMA out;
          host upcasts to f32.

Host-side marshaling: shard rows across 8 cores, pre-transpose c to
(128, 2, rows) fp8 pairs for DoubleRow, precompute h0n (0.008% of FLOPs).
"""

import os
import sys
from contextlib import ExitStack

import numpy as np
import ml_dtypes

for _p in ("/opt/trn_rl_repo", "/root/.axon_site/_ro/trn_rl_repo"):
    if os.path.isdir(_p) and _p not in sys.path:
        sys.path.insert(0, _p)

from concourse import bacc, mybir, tile  # noqa: E402
from concourse.bass_utils import run_bass_kernel_spmd  # noqa: E402

B_TOTAL = 131072
N_CORES = 8
B_CORE = B_TOTAL // N_CORES  # 16384
M1 = 256          # cepstral coefficients per row (M+1)
N_OUT = 512       # impulse response length
L = 512           # DFT length
NB = L // 2       # matmul-handled bins 1..NB (bin 0 via h0n slot)
NBC = NB // 128   # bin chunks (2)
ST_ROWS = 512     # rows per supertile
N_ST = B_CORE // ST_ROWS  # 32
S_C = 0.01        # host scale for c -> fp8

F32 = mybir.dt.float32
F16 = mybir.dt.float16
F8 = mybir.dt.float8e4
NP_F8 = ml_dtypes.float8_e4m3

_cache: dict = {}

TWO_PI = 2.0 * np.pi


def _install_sin2pi_patches():
    """Keep all activations in ONE ACT table set (exp_and_friends = {exp,
    sin2pi}) to avoid per-supertile table reloads (~2.7us each).

    1. Patch bacc's activation-table map so Exp and Sin both resolve to
       exp_and_friends -> bacc emits a single LoadActFuncSet.
    2. Rewrite "Sin" -> "Sin2pi" in the BIR json just before walrus; the
       kernel emits Sin with scale=S_C/(2*pi) so the arguments are already
       in sin2pi's convention (sin2pi(x) = sin(2*pi*x)).
    """
    if _cache.get("patched"):
        return
    import concourse.bacc as _bacc
    import concourse.bass2jax as _b2j

    SIN = mybir.ActivationFunctionType.Sin
    EXP = mybir.ActivationFunctionType.Exp
    _orig_tables = _bacc.get_activation_tables

    def tables_patched(arch):
        t = {k: set(v) for k, v in _orig_tables(arch).items()}
        for k in t:
            t[k].discard(SIN)
            if k != "exp_and_friends":
                t[k].discard(EXP)
        if "exp_and_friends" in t:
            t["exp_and_friends"] |= {SIN, EXP}
        return t

    _bacc.get_activation_tables = tables_patched

    _orig_compile = _b2j.compile_bir_kernel

    def compile_patched(bir_json, *a, **kw):
        # only rewrite THIS kernel's module (identified by its weight tensor)
        if isinstance(bir_json, bytes):
            if b'"wcf"' in bir_json:
                bir_json = bir_json.replace(b'"func":"Sin"', b'"func":"Sin2pi"')
        elif '"wcf"' in bir_json:
            bir_json = bir_json.replace('"func":"Sin"', '"func":"Sin2pi"')
        return _orig_compile(bir_json, *a, **kw)

    _b2j.compile_bir_kernel = compile_patched
    _cache["patched"] = True


def _host_weights():
    d = np.arange(M1, dtype=np.float64)
    k = np.arange(1, NB + 1, dtype=np.float64)
    th = 2.0 * np.pi * np.outer(d, k) / L           # (256, NB)
    wc = np.cos(th)
    ws = -np.sin(th)
    # DoubleRow pairing: [p, i, bin] holds coefficient d = i*128 + p
    wc8 = np.asarray(wc.reshape(2, 128, NB).transpose(1, 0, 2), dtype=NP_F8)
    ws8 = np.asarray(ws.reshape(2, 128, NB).transpose(1, 0, 2), dtype=NP_F8)
    n = np.arange(N_OUT, dtype=np.float64)
    thi = 2.0 * np.pi * np.outer(k, n) / L          # (NB, 512)
    w = np.where(k == NB, 1.0, 2.0)[:, None] / L
    ci = (w * np.cos(thi)).astype(np.float16)
    si = (-w * np.sin(thi)).astype(np.float16)
    assert np.all(si[NB - 1] == 0)
    si[NB - 1, :] = 1.0                              # h0n slot
    return wc8, ws8, ci, si


def _build(n_st=N_ST, repeat=1):
    _install_sin2pi_patches()
    nc = bacc.Bacc(
        "TRN2", target_bir_lowering=False, debug=False, num_devices=N_CORES
    )
    ct_ap = nc.dram_tensor("ct", [128, 2, n_st * ST_ROWS], F8, kind="ExternalInput").ap()
    h0_ap = nc.dram_tensor("h0n", [n_st, N_OUT], F16, kind="ExternalInput").ap()
    wc_ap = nc.dram_tensor("wcf", [128, 2, NB], F8, kind="ExternalInput").ap()
    ws_ap = nc.dram_tensor("wsf", [128, 2, NB], F8, kind="ExternalInput").ap()
    ci_ap = nc.dram_tensor("cif", [NB, N_OUT], F16, kind="ExternalInput").ap()
    si_ap = nc.dram_tensor("sif", [NB, N_OUT], F16, kind="ExternalInput").ap()
    h_ap = nc.dram_tensor("h", [n_st * ST_ROWS, N_OUT], F16, kind="ExternalOutput").ap()

    EXP = mybir.ActivationFunctionType.Exp
    SIN = mybir.ActivationFunctionType.Sin
    DR = mybir.MatmulPerfMode.DoubleRow

    with tile.TileContext(nc) as tc, ExitStack() as ctx:
        const = ctx.enter_context(tc.tile_pool(name="const", bufs=1))
        ctp = ctx.enter_context(tc.tile_pool(name="ctp", bufs=4))
        actp = ctx.enter_context(tc.tile_pool(name="actp", bufs=3))
        hp = ctx.enter_context(tc.tile_pool(name="hp", bufs=3))
        outp = ctx.enter_context(tc.tile_pool(name="outp", bufs=8))
        ps_ri = ctx.enter_context(tc.tile_pool(name="ps_ri", bufs=1, space="PSUM"))
        ps_h = ctx.enter_context(tc.tile_pool(name="ps_h", bufs=4, space="PSUM"))

        # constants
        wc_sb = const.tile([128, 2, NB], F8, tag="wc8")
        ws_sb = const.tile([128, 2, NB], F8, tag="ws8")
        nc.sync.dma_start(wc_sb[:], wc_ap[:])
        nc.sync.dma_start(ws_sb[:], ws_ap[:])
        ci_sb = [const.tile([128, N_OUT], F16, tag=f"ci{b}", name=f"ci{b}") for b in range(NBC)]
        si_sb = [const.tile([128, N_OUT], F16, tag=f"si{b}", name=f"si{b}") for b in range(NBC)]
        for b in range(NBC):
            nc.sync.dma_start(ci_sb[b][:], ci_ap[b * 128:(b + 1) * 128, :])
            nc.sync.dma_start(si_sb[b][:], si_ap[b * 128:(b + 1) * 128, :])
        zb = const.tile([128, 1], F32, tag="zb")
        nc.gpsimd.memset(zb[:], 0.0)
        quarter = const.tile([128, 1], F32, tag="quarter")
        nc.gpsimd.memset(quarter[:], 0.25)

        def fwd(st):
            """Load c chunk for supertile st and run the forward DFT."""
            r0 = st * ST_ROWS
            ct8 = ctp.tile([128, 2, ST_ROWS], F8, tag="ct8", name="ct8")
            nc.sync.dma_start(ct8[:], ct_ap[:, :, r0:r0 + ST_ROWS])
            r_ps = ps_ri.tile([128, 2 * ST_ROWS], F32, tag="R", name="r_ps")
            i_ps = ps_ri.tile([128, 2 * ST_ROWS], F32, tag="I", name="i_ps")
            for bc in range(NBC):
                nc.tensor.matmul(
                    r_ps[:, bc * ST_ROWS:(bc + 1) * ST_ROWS],
                    wc_sb[:, :, bc * 128:(bc + 1) * 128], ct8[:],
                    start=True, stop=True, perf_mode=DR,
                )
                nc.tensor.matmul(
                    i_ps[:, bc * ST_ROWS:(bc + 1) * ST_ROWS],
                    ws_sb[:, :, bc * 128:(bc + 1) * 128], ct8[:],
                    start=True, stop=True, perf_mode=DR,
                )
            return r_ps, i_ps

        total = n_st * repeat
        nxt = fwd(0)
        for it in range(total):
            st = it % n_st
            r0 = st * ST_ROWS
            r_ps, i_ps = nxt

            # ---- pointwise: A=exp(R), Sn=sin(I), Cs=cos(I), H=A*(Cs,Sn) ----
            a_sb = actp.tile([128, 2 * ST_ROWS], F16, tag="A")
            sn_sb = actp.tile([128, 2 * ST_ROWS], F16, tag="Sn")
            cs_sb = actp.tile([128, 2 * ST_ROWS], F16, tag="Cs")
            nc.scalar.activation(a_sb[:], r_ps[:], EXP, bias=zb[:], scale=float(S_C))
            nc.scalar.activation(sn_sb[:], i_ps[:], SIN, bias=zb[:],
                                 scale=float(S_C / TWO_PI))
            nc.scalar.activation(cs_sb[:], i_ps[:], SIN, bias=quarter[:],
                                 scale=float(S_C / TWO_PI))
            hre = hp.tile([128, 2 * ST_ROWS], F16, tag="HRe")
            him = hp.tile([128, 2 * ST_ROWS], F16, tag="HIm")
            nc.vector.tensor_mul(him[:], a_sb[:], sn_sb[:])
            nc.vector.tensor_mul(hre[:], a_sb[:], cs_sb[:])
            # inject per-row DC term into the (zero) Nyquist sin slot
            nc.sync.dma_start(him[127:128, ST_ROWS:2 * ST_ROWS], h0_ap[st:st + 1, :])

            # next supertile's forward overlaps this one's inverse
            if it + 1 < total:
                nxt = fwd((it + 1) % n_st)

            # ---- inverse DFT per row-chunk + store ----
            for rc in range(4):
                h_ps = ps_h.tile([128, N_OUT], F32, tag="h_ps")
                for bc in range(NBC):
                    c0 = bc * ST_ROWS + rc * 128
                    nc.tensor.matmul(
                        h_ps[:], hre[:, c0:c0 + 128], ci_sb[bc][:],
                        start=(bc == 0), stop=False,
                    )
                    nc.tensor.matmul(
                        h_ps[:], him[:, c0:c0 + 128], si_sb[bc][:],
                        start=False, stop=(bc == NBC - 1),
                    )
                o_sb = outp.tile([128, N_OUT], F16, tag="o_sb")
                if rc < 2:
                    nc.scalar.copy(o_sb[:], h_ps[:])
                else:
                    nc.vector.tensor_copy(o_sb[:], h_ps[:])
                nc.sync.dma_start(
                    h_ap[r0 + rc * 128: r0 + (rc + 1) * 128, :], o_sb[:]
                )

    nc.compile()
    return nc


def _get_nc(n_st=N_ST):
    key = ("nc", n_st)
    if key not in _cache:
        _cache[key] = _build(n_st)
    return _cache[key]


def _marshal(c_shard):
    """Host-side input marshaling for one core's row shard."""
    ct = np.ascontiguousarray(c_shard.T / S_C)                 # (256, rows)
    ct8 = np.asarray(
        ct.reshape(2, 128, -1).transpose(1, 0, 2), dtype=NP_F8
    )                                                          # (128, 2, rows)
    s0 = c_shard.astype(np.float64).sum(axis=1)                # (rows,)
    h0n = (np.exp(s0) / L).astype(np.float16)
    n_st = c_shard.shape[0] // ST_ROWS
    return np.ascontiguousarray(ct8), h0n.reshape(n_st, ST_ROWS)


def _in_maps(c):
    wc8, ws8, ci, si = _host_weights()
    maps = []
    for i in range(N_CORES):
        ct8, h0n = _marshal(c[i * B_CORE:(i + 1) * B_CORE])
        maps.append({
            "ct": ct8, "h0n": h0n,
            "wcf": wc8, "wsf": ws8, "cif": ci, "sif": si,
        })
    return maps


def kernel(c):
    c = np.ascontiguousarray(np.asarray(c), dtype=np.float32)
    assert c.shape == (B_TOTAL, M1), c.shape
    nc = _get_nc()
    res = run_bass_kernel_spmd(nc, _in_maps(c), list(range(N_CORES)))
    return np.concatenate(
        [res.results[i]["h"].astype(np.float32) for i in range(N_CORES)], axis=0
    )


# revision 12
# speedup vs baseline: 3.3198x; 3.3198x over previous
"""Cepstrum -> minimum-phase impulse response on 8 Trainium2 NeuronCores.

Math: the reference recurrence  n*h_n = sum_k (k c_k) h_{n-k}, h_0 = exp(c_0)
is exactly the power-series exponential h = exp(C(z)) mod z^512 for the
degree-255 polynomial C. We evaluate it spectrally with a length-512 DFT:

    h = IDFT_512( exp( DFT_512(c) ) )

Aliasing folds h_{n+512} into h_n; since order-2 products of C reach only
degree 510, the alias is order-3 (~1e-4 abs, validated on host in f64).

All four matmul groups run in fp8 DoubleRow mode (K=256 contraction per
pass, 2 fp8 weights/cell).  fp8's ~2% quantization forces one trick: the
inverse-DFT stationary operand is the CENTERED spectrum alpha*(H - 1), so
quantization error rides the O(0.15) fluctuation instead of the O(1)
background.  The subtracted background IDFT(1) = delta_n is restored on the
host (column 0 += 1); the -1/L DC residue and the true DC bin fold into the
h0n slot below.

Device pipeline per 512-row supertile:
  - fwd:  R/I[bins, rows] via one fp8 DoubleRow matmul per 128-bin chunk
  - ACT:  A = 16*exp(R) (alpha via bias=ln 16), Sn = sin(I), Cs = cos(I)
          (single table set: exp_and_friends, Sin lowered to Sin2pi)
  - DVE:  u = A*Cs (fp16), HRe8 = u - 16 (fp8), HIm8 = A*Sn (fp8)
  - the per-row DC term rides a free matmul slot: the Nyquist-bin sin row
    is identically zero, so si8[Nyq,:] := 1 and HIm8[Nyq,:] := DMA'd
    16384*(h0n - 1/512) where h0n = exp(sum c)/512
  - inv:  h[rows, n] via 2 DoubleRow matmuls per 128-row chunk over
    beta-scaled ci/si (beta=1024); psum = 16384*(h - delta)
  - out:  plain PSUM -> fp16 copy (split scalar/vector engines), DMA out;
    host upcasts, divides by 16384 and adds the delta column.

Host-side marshaling: shard rows across 8 cores, pre-transpose c to
(128, 2, rows) fp8 pairs for DoubleRow, precompute h0n (0.008% of FLOPs).
"""

import os
import sys
from contextlib import ExitStack

import numpy as np
import ml_dtypes

for _p in ("/opt/trn_rl_repo", "/root/.axon_site/_ro/trn_rl_repo"):
    if os.path.isdir(_p) and _p not in sys.path:
        sys.path.insert(0, _p)

from concourse import bacc, mybir, tile  # noqa: E402
from concourse.bass_utils import run_bass_kernel_spmd  # noqa: E402

B_TOTAL = 131072
N_CORES = 8
B_CORE = B_TOTAL // N_CORES  # 16384
M1 = 256          # cepstral coefficients per row (M+1)
N_OUT = 512       # impulse response length
L = 512           # DFT length
NB = L // 2       # matmul-handled bins 1..NB (bin 0 via h0n slot)
NBC = NB // 128   # bin chunks (2)
ST_ROWS = 512     # rows per supertile
N_ST = B_CORE // ST_ROWS  # 32
S_C = 0.01        # host scale for c -> fp8
ALPHA = 16.0      # fp8 scale for the centered spectrum
BETA = 1024.0     # fp8 scale for ci/si
AB = ALPHA * BETA

F32 = mybir.dt.float32
F16 = mybir.dt.float16
F8 = mybir.dt.float8e4
NP_F8 = ml_dtypes.float8_e4m3

_cache: dict = {}

TWO_PI = 2.0 * np.pi


def _install_sin2pi_patches():
    """Keep all activations in ONE ACT table set (exp_and_friends = {exp,
    sin2pi}) to avoid per-supertile table reloads (~2.7us each).

    1. Patch bacc's activation-table map so Exp and Sin both resolve to
       exp_and_friends -> bacc emits a single LoadActFuncSet.
    2. Rewrite "Sin" -> "Sin2pi" in the BIR json just before walrus; the
       kernel emits Sin with scale=S_C/(2*pi) so the arguments are already
       in sin2pi's convention (sin2pi(x) = sin(2*pi*x)).
    """
    if _cache.get("patched"):
        return
    import concourse.bacc as _bacc
    import concourse.bass2jax as _b2j

    SIN = mybir.ActivationFunctionType.Sin
    EXP = mybir.ActivationFunctionType.Exp
    _orig_tables = _bacc.get_activation_tables

    def tables_patched(arch):
        t = {k: set(v) for k, v in _orig_tables(arch).items()}
        for k in t:
            t[k].discard(SIN)
            if k != "exp_and_friends":
                t[k].discard(EXP)
        if "exp_and_friends" in t:
            t["exp_and_friends"] |= {SIN, EXP}
        return t

    _bacc.get_activation_tables = tables_patched

    _orig_compile = _b2j.compile_bir_kernel

    def compile_patched(bir_json, *a, **kw):
        # only rewrite THIS kernel's module (identified by its weight tensor)
        if isinstance(bir_json, bytes):
            if b'"wcf"' in bir_json:
                bir_json = bir_json.replace(b'"func":"Sin"', b'"func":"Sin2pi"')
        elif '"wcf"' in bir_json:
            bir_json = bir_json.replace('"func":"Sin"', '"func":"Sin2pi"')
        return _orig_compile(bir_json, *a, **kw)

    _b2j.compile_bir_kernel = compile_patched
    _cache["patched"] = True


def _pair(x):
    """(256, F) -> (128, 2, F) DoubleRow pairing: [p, i] holds row i*128+p."""
    return np.ascontiguousarray(x.reshape(2, 128, -1).transpose(1, 0, 2))


def _host_weights():
    d = np.arange(M1, dtype=np.float64)
    k = np.arange(1, NB + 1, dtype=np.float64)
    th = 2.0 * np.pi * np.outer(d, k) / L           # (256, NB)
    wc8 = np.asarray(_pair(np.cos(th)), dtype=NP_F8)
    ws8 = np.asarray(_pair(-np.sin(th)), dtype=NP_F8)
    n = np.arange(N_OUT, dtype=np.float64)
    thi = 2.0 * np.pi * np.outer(k, n) / L          # (NB, 512)
    w = np.where(k == NB, 1.0, 2.0)[:, None] / L
    ci8 = np.asarray(_pair(BETA * w * np.cos(thi)), dtype=NP_F8)
    si = _pair(-BETA * w * np.sin(thi))
    assert np.abs(si[127, 1]).max() < 1e-9          # Nyquist sin row is ~0
    si[127, 1, :] = 1.0                              # h0n slot (Nyquist sin)
    si8 = np.asarray(si, dtype=NP_F8)
    return wc8, ws8, ci8, si8


def _build(n_st=N_ST, repeat=1):
    _install_sin2pi_patches()
    nc = bacc.Bacc(
        "TRN2", target_bir_lowering=False, debug=False, num_devices=N_CORES
    )
    ct_ap = nc.dram_tensor("ct", [128, 2, n_st * ST_ROWS], F8, kind="ExternalInput").ap()
    h0_ap = nc.dram_tensor("h0n", [n_st, N_OUT], F8, kind="ExternalInput").ap()
    wc_ap = nc.dram_tensor("wcf", [128, 2, NB], F8, kind="ExternalInput").ap()
    ws_ap = nc.dram_tensor("wsf", [128, 2, NB], F8, kind="ExternalInput").ap()
    ci_ap = nc.dram_tensor("cif", [128, 2, N_OUT], F8, kind="ExternalInput").ap()
    si_ap = nc.dram_tensor("sif", [128, 2, N_OUT], F8, kind="ExternalInput").ap()
    h_ap = nc.dram_tensor("h", [n_st * ST_ROWS, N_OUT], F16, kind="ExternalOutput").ap()

    EXP = mybir.ActivationFunctionType.Exp
    SIN = mybir.ActivationFunctionType.Sin
    DR = mybir.MatmulPerfMode.DoubleRow

    with tile.TileContext(nc) as tc, ExitStack() as ctx:
        const = ctx.enter_context(tc.tile_pool(name="const", bufs=1))
        ctp = ctx.enter_context(tc.tile_pool(name="ctp", bufs=4))
        actp = ctx.enter_context(tc.tile_pool(name="actp", bufs=3))
        hp = ctx.enter_context(tc.tile_pool(name="hp", bufs=3))
        outp = ctx.enter_context(tc.tile_pool(name="outp", bufs=8))
        ps_ri = ctx.enter_context(tc.tile_pool(name="ps_ri", bufs=1, space="PSUM"))
        ps_h = ctx.enter_context(tc.tile_pool(name="ps_h", bufs=4, space="PSUM"))

        # constants
        wc_sb = const.tile([128, 2, NB], F8, tag="wc8")
        ws_sb = const.tile([128, 2, NB], F8, tag="ws8")
        nc.sync.dma_start(wc_sb[:], wc_ap[:])
        nc.sync.dma_start(ws_sb[:], ws_ap[:])
        ci_sb = const.tile([128, 2, N_OUT], F8, tag="ci8")
        si_sb = const.tile([128, 2, N_OUT], F8, tag="si8")
        nc.sync.dma_start(ci_sb[:], ci_ap[:])
        nc.sync.dma_start(si_sb[:], si_ap[:])
        zb = const.tile([128, 1], F32, tag="zb")
        nc.gpsimd.memset(zb[:], 0.0)
        quarter = const.tile([128, 1], F32, tag="quarter")
        nc.gpsimd.memset(quarter[:], 0.25)
        lna = const.tile([128, 1], F32, tag="lna")
        nc.gpsimd.memset(lna[:], float(np.log(ALPHA)))

        variant = os.environ.get("BASS_VARIANT", "")
        if variant in ("tensor_only", "inv_only"):
            hre_c = const.tile([128, 2, ST_ROWS], F8, tag="hre_c")
            him_c = const.tile([128, 2, ST_ROWS], F8, tag="him_c")
            nc.gpsimd.memset(hre_c[:], 1.0)
            nc.gpsimd.memset(him_c[:], 0.5)

        def fwd(st):
            """Load c chunk for supertile st and run the forward DFT."""
            r0 = st * ST_ROWS
            ct8 = ctp.tile([128, 2, ST_ROWS], F8, tag="ct8", name="ct8")
            nc.sync.dma_start(ct8[:], ct_ap[:, :, r0:r0 + ST_ROWS])
            if os.environ.get("BASS_VARIANT") == "inv_only":
                return None, None
            r_ps = ps_ri.tile([128, 2 * ST_ROWS], F32, tag="R", name="r_ps")
            i_ps = ps_ri.tile([128, 2 * ST_ROWS], F32, tag="I", name="i_ps")
            for bc in range(NBC):
                nc.tensor.matmul(
                    r_ps[:, bc * ST_ROWS:(bc + 1) * ST_ROWS],
                    wc_sb[:, :, bc * 128:(bc + 1) * 128], ct8[:],
                    start=True, stop=True, perf_mode=DR,
                )
                nc.tensor.matmul(
                    i_ps[:, bc * ST_ROWS:(bc + 1) * ST_ROWS],
                    ws_sb[:, :, bc * 128:(bc + 1) * 128], ct8[:],
                    start=True, stop=True, perf_mode=DR,
                )
            return r_ps, i_ps

        total = n_st * repeat
        nxt = fwd(0)
        for it in range(total):
            st = it % n_st
            r0 = st * ST_ROWS
            r_ps, i_ps = nxt

            if variant in ("tensor_only", "inv_only"):
                hre8, him8 = hre_c, him_c
            else:
                # ---- pointwise: A=16exp(R), Sn=sin(I), Cs=cos(I) ----
                a_sb = actp.tile([128, 2 * ST_ROWS], F16, tag="A")
                sn_sb = actp.tile([128, 2 * ST_ROWS], F16, tag="Sn")
                cs_sb = actp.tile([128, 2 * ST_ROWS], F16, tag="Cs")
                nc.scalar.activation(a_sb[:], r_ps[:], EXP, bias=lna[:],
                                     scale=float(S_C))
                nc.scalar.activation(sn_sb[:], i_ps[:], SIN, bias=zb[:],
                                     scale=float(S_C / TWO_PI))
                nc.scalar.activation(cs_sb[:], i_ps[:], SIN, bias=quarter[:],
                                     scale=float(S_C / TWO_PI))
                # centered fp8 spectrum: HRe8 = A*Cs - 16, HIm8 = A*Sn
                u_sb = hp.tile([128, 2 * ST_ROWS], F16, tag="U")
                hre8 = hp.tile([128, 2, ST_ROWS], F8, tag="HRe")
                him8 = hp.tile([128, 2, ST_ROWS], F8, tag="HIm")
                nc.vector.tensor_mul(him8[:], a_sb[:], sn_sb[:])
                nc.vector.tensor_mul(u_sb[:], a_sb[:], cs_sb[:])
                nc.vector.tensor_scalar_sub(hre8[:], u_sb[:], float(ALPHA))
                # inject per-row DC term into the (zero) Nyquist sin slot
                nc.sync.dma_start(him8[127:128, 1:2, :], h0_ap[st:st + 1, :])

            # next supertile's forward overlaps this one's inverse
            if it + 1 < total:
                nxt = fwd((it + 1) % n_st)

            # ---- inverse DFT per row-chunk + store ----
            for rc in range(4):
                h_ps = ps_h.tile([128, N_OUT], F32, tag="h_ps")
                nc.tensor.matmul(
                    h_ps[:], hre8[:, :, rc * 128:(rc + 1) * 128], ci_sb[:],
                    start=True, stop=False, perf_mode=DR,
                )
                nc.tensor.matmul(
                    h_ps[:], him8[:, :, rc * 128:(rc + 1) * 128], si_sb[:],
                    start=False, stop=True, perf_mode=DR,
                )
                o_sb = outp.tile([128, N_OUT], F16, tag="o_sb")
                if rc < 2:
                    nc.scalar.copy(o_sb[:], h_ps[:])
                else:
                    nc.vector.tensor_copy(o_sb[:], h_ps[:])
                nc.sync.dma_start(
                    h_ap[r0 + rc * 128: r0 + (rc + 1) * 128, :], o_sb[:]
                )

    nc.compile()
    return nc


def _get_nc(n_st=N_ST):
    key = ("nc", n_st)
    if key not in _cache:
        _cache[key] = _build(n_st)
    return _cache[key]


def _marshal(c_shard):
    """Host-side input marshaling for one core's row shard."""
    ct8 = np.asarray(_pair(c_shard.T / S_C), dtype=NP_F8)      # (128, 2, rows)
    s0 = c_shard.astype(np.float64).sum(axis=1)                # (rows,)
    h0n = np.asarray(AB * (np.exp(s0) / L - 1.0 / L), dtype=NP_F8)
    n_st = c_shard.shape[0] // ST_ROWS
    return ct8, np.ascontiguousarray(h0n.reshape(n_st, ST_ROWS))


def _in_maps(c):
    wc8, ws8, ci8, si8 = _host_weights()
    maps = []
    for i in range(N_CORES):
        ct8, h0n = _marshal(c[i * B_CORE:(i + 1) * B_CORE])
        maps.append({
            "ct": ct8, "h0n": h0n,
            "wcf": wc8, "wsf": ws8, "cif": ci8, "sif": si8,
        })
    return maps


def kernel(c):
    c = np.ascontiguousarray(np.asarray(c), dtype=np.float32)
    assert c.shape == (B_TOTAL, M1), c.shape
    nc = _get_nc()
    res = run_bass_kernel_spmd(nc, _in_maps(c), list(range(N_CORES)))
    out = np.concatenate(
        [res.results[i]["h"].astype(np.float32) for i in range(N_CORES)], axis=0
    )
    out *= 1.0 / AB
    out[:, 0] += 1.0     # restore the IDFT(1) = delta_n background
    return out
